# revision 54
# baseline (speedup 1.0000x reference)
"""Trainium2 Bass kernel for a 2-layer directed GCN (PyG GCNConv semantics).

Strategy (8-core SPMD, 1D node sharding):
  - Nodes sharded across 8 cores (12500 each, padded to 12544 = 98*128).
  - Per-edge coefficients nrm = dinv[row]*ew*dinv[col] and self-loop weights
    dinv^2 are host-precomputed (graph-structure preprocessing, cacheable).
  - Linearity trick: aggregate raw features first, apply W afterwards:
        out[c] = (sum_e nrm_e * x[row_e] + dinv_c^2 * x_c) @ W + b
  - Transposed aggregation: per 128-edge tile, matmul(psT, lhsT=gt, rhs=S)
    accumulates psT[f, dest] = sum_e gt[e, f] * S[e, dest] in PSUM, where
    S[e, d] = (iota==col_e)*nrm_e is built by one fused DVE tensor_scalar.
  - Layer 1: edges partitioned by destination core, cells = (chunk, dtile)
    with 4 source windows for int16 gather reach; gathers raw bf16 x from a
    replicated table (no collective).
  - The relu output h is stored ONLY as a packed fp8-e4m3 slab (12544x128B,
    written straight from PSUM by the relu activation). The AllGather moves
    fp8 - half the bf16 payload - split in two (AG_A after SPLIT_S
    supertiles of layer 1, AG_B at the end) to overlap with compute.
  - All layer-2 aggregation passes gather PAIRS of fp8 rows (elem = 256B =
    rows 2k,2k+1 of a packed fp8 table; dma_gather requires elem % 256B ==
    0). Pair indexing halves the index space (each table fits ONE int16
    window) and cells are whole supertiles -> little tile padding. Parity
    (row 2k vs 2k+1) and multi-dtile routing are resolved by a single
    widened one-hot build: colw2 = col + 128*(2*(dtile-d0) + parity), S2 =
    (iota_w == colw2) * nrm over [128, span*256], one fp8 x bf16 matmul per
    populated (dtile, parity) slice (mixed-dtype matmul keeps nrm exact in
    bf16; fp8 nrm would cost ~2x the error).
  - Layer-2 passes: local (own-slab sources, no collective dependency) and
    remote pass A (sources in the AG_A half) run inside the AG_B window,
    folding psums into bf16 SBUF accumulators; pass B runs after AG_B and
    merges acc via an extra W2 matmul (accT@W2 + psum@W2 + bias in one PSUM
    group), and stores. The relu bf16 tiles stay RESIDENT in SBUF, so the
    layer-2 self-loop term reads bf16 with no DMA (fp8 self-rows would put
    fp8's ~5% relative error directly on low-degree outputs: selfw~1).
  - Batched supertile IO: xsel loads and h/out stores move 4 dest tiles
    per DMA to amortize per-DMA overheads.
"""

import ml_dtypes
import numpy as np

import concourse.bacc as bacc
import concourse.mybir as mybir
import concourse.tile as tile
from concourse.bass_utils import run_bass_kernel_spmd
from concourse.library_config import mlp

N_NODES = 100000
D = 128
N_CORES = 8
NPC = N_NODES // N_CORES          # 12500 nodes per core
TPC = (NPC + 127) // 128          # 98 destination tiles per core
PAD_NPC = TPC * 128               # 12544 padded nodes per core
N_PAD = N_CORES * PAD_NPC         # 100352 padded table rows
NCHUNK = 4
# Asymmetric chunks (each < 32768 for int16 gather reach).
CHUNK_SIZES = (27776, 27392, 28416, 16768)
CHUNK_LO = (0, 27776, 55168, 83584)
assert sum(CHUNK_SIZES) == N_PAD
SUPER = 4                         # dest tiles per supertile
# Split AllGather: rows [0, SPLIT_R) of each slab go in AG_A (issued after
# supertile SPLIT_S-1 of layer 1), the rest in AG_B. The layer-2 remote pass
# is split by source half so pass A overlaps AG_B.
SPLIT_D = 48                      # dtiles per core in half A
SPLIT_R = SPLIT_D * 128           # rows per core in half A (even!)
SPLIT_S = SPLIT_D // SUPER
NA_ROWS = 8 * SPLIT_R             # rows in fp8 table A
NB_ROWS = N_PAD - NA_ROWS        # rows in fp8 table B
assert NA_ROWS // 2 < 32768 and NB_ROWS // 2 < 32768  # one int16 window each

F32 = mybir.dt.float32
BF16 = mybir.dt.bfloat16
FP16 = mybir.dt.float16
F8 = mybir.dt.float8e4
I16 = mybir.dt.int16
NPBF = ml_dtypes.bfloat16
NSUP = (TPC + SUPER - 1) // SUPER
WMAX = 4 * 256                    # widest one-hot build (4 dtiles x 2 parity)


def _seg_tiles(t_counts):
    """t_counts[c][d] -> seg[s][c] = tiles in supertile s, chunk c."""
    nch = len(t_counts)
    return [[sum(t_counts[c][d]
                 for d in range(s * SUPER, min((s + 1) * SUPER, TPC)))
             for c in range(nch)] for s in range(NSUP)]


def _build_nc(seg1, meta1, segL, metaL, segA, metaA, segB, metaB):
    """seg1: [NSUP][NCHUNK] layer-1 gather tiles per (supertile, window);
    segL/segA/segB: [NSUP] pair-gather tiles per supertile (local /
    remote-A / remote-B); metaX: per-tile (d0, ((doff, parity), ...))
    matmul slice lists (cross-core union)."""
    NT1 = sum(sum(r) for r in seg1)
    NTL = sum(segL)
    NTRA = sum(segA)
    NTRB = sum(segB)
    nc = bacc.Bacc("TRN2", target_bir_lowering=False)

    # partition-major [128, TPC*128]: per-partition DMA runs are 1024B+,
    # dodging the <512B descriptor penalty (364ns vs 728ns per supertile).
    x_selT = nc.dram_tensor("x_selT", [128, TPC * 128], BF16,
                            kind="ExternalInput")
    x_tab = nc.dram_tensor("x_tab", [N_PAD, D], BF16, kind="ExternalInput")
    gix1 = nc.dram_tensor("gix1", [128, NT1 * 8], I16, kind="ExternalInput")
    colw1 = nc.dram_tensor("colw1", [128, NT1], F32, kind="ExternalInput")
    nrmw1 = nc.dram_tensor("nrmw1", [128, NT1], F32, kind="ExternalInput")
    gixRA = nc.dram_tensor("gixRA", [128, NTRA * 8], I16, kind="ExternalInput")
    colwRA = nc.dram_tensor("colwRA", [128, NTRA], F32, kind="ExternalInput")
    nrmwRA = nc.dram_tensor("nrmwRA", [128, NTRA], F32, kind="ExternalInput")
    gixRB = nc.dram_tensor("gixRB", [128, NTRB * 8], I16, kind="ExternalInput")
    colwRB = nc.dram_tensor("colwRB", [128, NTRB], F32, kind="ExternalInput")
    nrmwRB = nc.dram_tensor("nrmwRB", [128, NTRB], F32, kind="ExternalInput")
    gixL = nc.dram_tensor("gixL", [128, NTL * 8], I16, kind="ExternalInput")
    colwL = nc.dram_tensor("colwL", [128, NTL], F32, kind="ExternalInput")
    nrmwL = nc.dram_tensor("nrmwL", [128, NTL], F32, kind="ExternalInput")
    selfw = nc.dram_tensor("selfw", [128, TPC], F32, kind="ExternalInput")
    pidx = nc.dram_tensor("pidx", [128, 1], F32, kind="ExternalInput")
    w1 = nc.dram_tensor("w1", [D, D], BF16, kind="ExternalInput")
    b1 = nc.dram_tensor("b1", [1, D], BF16, kind="ExternalInput")
    w2 = nc.dram_tensor("w2", [D, D], BF16, kind="ExternalInput")
    b2 = nc.dram_tensor("b2", [1, D], BF16, kind="ExternalInput")
    onesb = nc.dram_tensor("onesb", [1, D], BF16, kind="ExternalInput")
    iota = nc.dram_tensor("iota", [128, 128], BF16, kind="ExternalInput")
    iota_w = nc.dram_tensor("iota_w", [128, WMAX], FP16, kind="ExternalInput")
    ones = nc.dram_tensor("ones", [1, D], F32, kind="ExternalInput")
    out_slab = nc.dram_tensor("out_slab", [128, TPC * 128], BF16,
                              kind="ExternalOutput")

    max_seg1 = max(max(r) for r in seg1) or 1
    max_segR = max(max(segA), max(segB), max(segL)) or 1

    with tile.TileContext(nc) as tc:
        nc.gpsimd.load_library(mlp)
        with (
            tc.tile_pool(name="const", bufs=1) as constp,
            tc.tile_pool(name="gbuf", bufs=4) as gbufp,
            tc.tile_pool(name="gbufR", bufs=3) as gbufRp,
            tc.tile_pool(name="sbld", bufs=8) as sbldp,
            tc.tile_pool(name="sbldw", bufs=8) as sbldwp,
            tc.tile_pool(name="selb", bufs=2) as selbp,
            tc.tile_pool(name="obres", bufs=1) as obresp,
            tc.tile_pool(name="accp", bufs=1) as accp,
            tc.tile_pool(name="tailp", bufs=6) as tailp,
            tc.tile_pool(name="obat", bufs=3) as obatp,
            tc.tile_pool(name="psag", bufs=6, space="PSUM") as psagp,
            tc.tile_pool(name="pso", bufs=2, space="PSUM") as psop,
            tc.tile_pool(name="dram", bufs=1, space="DRAM") as dramp,
        ):
            gix1_s = constp.tile([128, NT1 * 8], I16, tag="gix1")
            colw1_s = constp.tile([128, NT1], F32, tag="colw1")
            nrmw1_s = constp.tile([128, NT1], F32, tag="nrmw1")
            gixRA_s = constp.tile([128, NTRA * 8], I16, tag="gixRA")
            colwRA_s = constp.tile([128, NTRA], F32, tag="colwRA")
            nrmwRA_s = constp.tile([128, NTRA], F32, tag="nrmwRA")
            gixRB_s = constp.tile([128, NTRB * 8], I16, tag="gixRB")
            colwRB_s = constp.tile([128, NTRB], F32, tag="colwRB")
            nrmwRB_s = constp.tile([128, NTRB], F32, tag="nrmwRB")
            gixL_s = constp.tile([128, NTL * 8], I16, tag="gixL")
            colwL_s = constp.tile([128, NTL], F32, tag="colwL")
            nrmwL_s = constp.tile([128, NTL], F32, tag="nrmwL")
            selfw_s = constp.tile([128, TPC], F32, tag="selfw")
            pidx_s = constp.tile([128, 1], F32, tag="pidx")
            w1_s = constp.tile([D, D], BF16, tag="w1")
            b1_s = constp.tile([1, D], BF16, tag="b1")
            w2_s = constp.tile([D, D], BF16, tag="w2")
            b2_s = constp.tile([1, D], BF16, tag="b2")
            onesb_s = constp.tile([1, D], BF16, tag="onesb")
            iota_s = constp.tile([128, 128], BF16, tag="iota")
            iotaw_s = constp.tile([128, WMAX], FP16, tag="iota_w")
            ones_s = constp.tile([1, D], F32, tag="ones")
            q = NT1 // 4
            for lo_t, hi_t in ((0, q), (q, 2 * q), (2 * q, 3 * q),
                               (3 * q, NT1)):
                nc.sync.dma_start(gix1_s[:, lo_t * 8:hi_t * 8],
                                  gix1[:, lo_t * 8:hi_t * 8])
            for dst, srct in ((colw1_s, colw1), (nrmw1_s, nrmw1),
                              (iota_s, iota), (iotaw_s, iota_w),
                              (pidx_s, pidx),
                              (selfw_s, selfw), (w1_s, w1), (b1_s, b1),
                              (ones_s, ones), (onesb_s, onesb)):
                nc.sync.dma_start(dst[:], srct[:])
            with tc.tile_wait_until(1.8):
                for dst, srct in ((gixL_s, gixL), (colwL_s, colwL),
                                  (nrmwL_s, nrmwL), (w2_s, w2),
                                  (b2_s, b2)):
                    nc.sync.dma_start(dst[:], srct[:])

            h_loc_f8 = dramp.tile([PAD_NPC, D], F8, tag="h_loc_f8")
            h_fullA8 = dramp.tile([NA_ROWS, D], F8, tag="h_fullA8",
                                  addr_space="Shared")
            h_fullB8 = dramp.tile([NB_ROWS, D], F8, tag="h_fullB8",
                                  addr_space="Shared")
            winL = h_loc_f8.rearrange("(n two) d -> n (two d)", two=2)
            winA = h_fullA8.rearrange("(n two) d -> n (two d)", two=2)
            winB = h_fullB8.rearrange("(n two) d -> n (two d)", two=2)

            def diag_tile(d, pool):
                """[128,128] diag(selfw[:, d]) via (iota==pidx)*selfw."""
                s_t = pool.tile([128, 128], BF16, tag="sbld")
                nc.vector.tensor_scalar(
                    s_t[:], iota_s[:], pidx_s[:, 0:1], selfw_s[:, d:d + 1],
                    mybir.AluOpType.is_equal, mybir.AluOpType.mult)
                return s_t

            def one_hot(colw_s, nrmw_s, tt, pool):
                s_t = pool.tile([128, 128], BF16, tag="sbld")
                nc.vector.tensor_scalar(
                    s_t[:], iota_s[:], colw_s[:, tt:tt + 1],
                    nrmw_s[:, tt:tt + 1],
                    mybir.AluOpType.is_equal, mybir.AluOpType.mult)
                return s_t

            def one_hot_w(colw_s, nrmw_s, tt, width, eng=None):
                """Widened build: S2[e, :] = (iota_w == colw2_e) * nrm_e."""
                s_t = sbldwp.tile([128, WMAX], BF16, tag="sbldw")
                (eng or nc.vector).tensor_scalar(
                    s_t[:, :width], iotaw_s[:, :width], colw_s[:, tt:tt + 1],
                    nrmw_s[:, tt:tt + 1],
                    mybir.AluOpType.is_equal, mybir.AluOpType.mult)
                return s_t

            def sup_range(s):
                return list(range(s * SUPER, min((s + 1) * SUPER, TPC)))

            # ---------------- layer 1 ----------------
            obsel = {}           # resident bf16 relu tiles per supertile
            slot1 = 0            # running tile slot in the L1 tables
            for s in range(NSUP):
                dlist = sup_range(s)
                total_d = {}
                spans = []
                for c in range(NCHUNK):
                    seg = seg1[s][c]
                    spans.append((slot1, seg))
                    for t in range(seg):
                        d0t, sl = meta1[slot1 + t]
                        for (jj, _p) in sl:
                            total_d[d0t + jj] = total_d.get(d0t + jj, 0) + 1
                    slot1 += seg
                ps = {d: psagp.tile([128, 128], F32, tag="psag",
                                    name=f"ps1_{s}_{d}")
                      for d in dlist}
                done = {d: 0 for d in dlist}
                for c, (lo, seg) in enumerate(spans):
                    if seg == 0:
                        continue
                    gt = gbufp.tile([128, max_seg1, 128], BF16, tag="gbuf")
                    nc.gpsimd.dma_gather(
                        gt[:, :seg, :],
                        x_tab[CHUNK_LO[c]:CHUNK_LO[c] + CHUNK_SIZES[c], :],
                        gix1_s[:, lo * 8:(lo + seg) * 8],
                        seg * 128, seg * 128, D,
                        single_packet=False)
                    for t in range(seg):
                        tt = lo + t
                        d0t, sl = meta1[tt]
                        if not sl:
                            continue
                        width = (max(jj for jj, _ in sl) + 1) * 128
                        s_t = one_hot_w(colw1_s, nrmw1_s, tt, width)
                        for (jj, _p) in sl:
                            d = d0t + jj
                            nc.tensor.matmul(
                                ps[d][:], gt[:, t, :],
                                s_t[:, jj * 128:(jj + 1) * 128],
                                start=(done[d] == 0), stop=False)
                            done[d] += 1
                # batched self-source load: x rows for this supertile
                nsd = len(dlist)
                xsel = selbp.tile([128, SUPER, 128], BF16, tag="xsel")
                nc.sync.dma_start(
                    xsel[:, :nsd, :].opt(),
                    x_selT[:, dlist[0] * 128:(dlist[-1] + 1) * 128]
                    .rearrange("p (n d) -> p n d", d=128))
                ob = obresp.tile([128, SUPER * 128], BF16, tag=f"ob{s}",
                                 name=f"ob_{s}")
                ob8 = obatp.tile([128, SUPER * 128], F8, tag="obat8")
                for j, d in enumerate(dlist):
                    dg = diag_tile(d, sbldp)
                    nc.tensor.matmul(ps[d][:], xsel[:, j, :], dg[:],
                                     start=(total_d.get(d, 0) == 0),
                                     stop=True)
                    aggT = tailp.tile([128, 128], BF16, tag="aggT")
                    nc.scalar.activation(
                        aggT[:], ps[d][:], mybir.ActivationFunctionType.Copy)
                    ps_o = psop.tile([128, 128], F32, tag="pso")
                    nc.tensor.matmul(ps_o[:], aggT[:], w1_s[:],
                                     start=True, stop=False)
                    nc.tensor.matmul(ps_o[:], onesb_s[:], b1_s[:],
                                     start=False, stop=True)
                    nc.scalar.activation(ob[:, j * 128:(j + 1) * 128],
                                         ps_o[:],
                                         mybir.ActivationFunctionType.Relu)
                nc.scalar.activation(ob8[:, :nsd * 128], ob[:, :nsd * 128],
                                     mybir.ActivationFunctionType.Copy)
                obsel[s] = ob
                nc.sync.dma_start(
                    h_loc_f8[dlist[0] * 128:(dlist[-1] + 1) * 128, :]
                    .rearrange("(n p) d -> p n d", p=128),
                    ob8[:, :nsd * 128].rearrange("p (n d) -> p n d", d=128))
                if s == SPLIT_S - 1:
                    # first-half AllGather: overlaps the rest of layer 1
                    nc.gpsimd.collective_compute(
                        "AllGather", mybir.AluOpType.bypass,
                        replica_groups=[list(range(N_CORES))],
                        ins=[h_loc_f8[0:SPLIT_R, :].opt()],
                        outs=[h_fullA8.opt()])

            # ---------------- second-half AllGather -------------------------
            nc.gpsimd.collective_compute(
                "AllGather", mybir.AluOpType.bypass,
                replica_groups=[list(range(N_CORES))],
                ins=[h_loc_f8[SPLIT_R:PAD_NPC, :].opt()],
                outs=[h_fullB8.opt()])

            accT = {}

            def fold_pass(seg_t, meta, gix_s, colw_s, nrmw_s, win, pname):
                """Pair-gather aggregation pass folding psums into accT."""
                slotR = 0
                for s in range(NSUP):
                    seg = seg_t[s]
                    if seg == 0:
                        continue
                    total_d = {}
                    for t in range(seg):
                        d0t, sl = meta[slotR + t]
                        for (jj, p) in sl:
                            total_d[d0t + jj] = total_d.get(d0t + jj, 0) + 1
                    ps = {d: psagp.tile([128, 128], F32, tag="psag",
                                        name=f"ps{pname}_{s}_{d}")
                          for d in total_d}
                    gt = gbufRp.tile([128, max_segR, 256], F8, tag="gbufR")
                    nc.gpsimd.dma_gather(
                        gt[:, :seg, :], win,
                        gix_s[:, slotR * 8:(slotR + seg) * 8],
                        seg * 128, seg * 128, 256,
                        single_packet=False)
                    done = {d: 0 for d in total_d}
                    for t in range(seg):
                        tt = slotR + t
                        d0t, sl = meta[tt]
                        if not sl:
                            continue
                        width = (max(jj for jj, _ in sl) + 1) * 256
                        s_t = one_hot_w(colw_s, nrmw_s, tt, width)
                        for (jj, p) in sl:
                            d = d0t + jj
                            nc.tensor.matmul(
                                ps[d][:], gt[:, t, p * 128:(p + 1) * 128],
                                s_t[:, (2 * jj + p) * 128:
                                     (2 * jj + p + 1) * 128],
                                start=(done[d] == 0),
                                stop=(done[d] == total_d[d] - 1))
                            done[d] += 1
                    slotR += seg
                    for d in sorted(ps):
                        if d in accT:
                            nc.vector.tensor_tensor(
                                accT[d][:], accT[d][:], ps[d][:],
                                mybir.AluOpType.add)
                        else:
                            a = accp.tile([128, 128], BF16, tag=f"accT{d}",
                                          name=f"accT_{d}")
                            nc.scalar.activation(
                                a[:], ps[d][:],
                                mybir.ActivationFunctionType.Copy)
                            accT[d] = a

            # ---------------- layer 2: local + remote pass A ---------------
            # Created after the AG_B instruction so they issue inside the
            # AG_B window. Local pass gathers the own fp8 slab (no
            # collective dependency); pass A gathers the AG_A table.
            wait_ctx = tc.tile_wait_until(1.0)
            wait_ctx.__enter__()
            fold_pass(segL, metaL, gixL_s, colwL_s, nrmwL_s, winL, "L")
            # remote-table consts: on the Pool queue after the local-pass
            # gathers, so they don't delay the first local gather but are
            # still gated behind AG_B's program order.
            for dst, srct in ((gixRA_s, gixRA), (colwRA_s, colwRA),
                              (nrmwRA_s, nrmwRA), (gixRB_s, gixRB),
                              (colwRB_s, colwRB), (nrmwRB_s, nrmwRB)):
                nc.gpsimd.dma_start(dst[:], srct[:])
            fold_pass(segA, metaA, gixRA_s, colwRA_s, nrmwRA_s, winA, "A")
            wait_ctx.__exit__(None, None, None)

            # ---------------- layer 2 remote pass B + tail ------------------
            wait_ctx = tc.tile_wait_until(2.0)
            wait_ctx.__enter__()
            slotR = 0
            for s in range(NSUP):
                dlist = sup_range(s)
                seg = segB[s]
                total_d = {}
                for t in range(seg):
                    d0t, sl = metaB[slotR + t]
                    for (jj, p) in sl:
                        total_d[d0t + jj] = total_d.get(d0t + jj, 0) + 1
                ps = {d: psagp.tile([128, 128], F32, tag="psag",
                                    name=f"psRB_{s}_{d}")
                      for d in dlist}
                if seg > 0:
                    gt = gbufRp.tile([128, max_segR, 256], F8, tag="gbufR")
                    nc.gpsimd.dma_gather(
                        gt[:, :seg, :], winB,
                        gixRB_s[:, slotR * 8:(slotR + seg) * 8],
                        seg * 128, seg * 128, 256,
                        single_packet=False)
                    done = {d: 0 for d in total_d}
                    for t in range(seg):
                        tt = slotR + t
                        d0t, sl = metaB[tt]
                        if not sl:
                            continue
                        width = (max(jj for jj, _ in sl) + 1) * 256
                        s_t = one_hot_w(colwRB_s, nrmwRB_s, tt, width)
                        for (jj, p) in sl:
                            d = d0t + jj
                            nc.tensor.matmul(
                                ps[d][:], gt[:, t, p * 128:(p + 1) * 128],
                                s_t[:, (2 * jj + p) * 128:
                                     (2 * jj + p + 1) * 128],
                                start=(done[d] == 0), stop=False)
                            done[d] += 1
                    slotR += seg
                ob2 = obatp.tile([128, SUPER * 128], BF16, tag="obat2")
                hs = obsel[s]
                nsd = len(dlist)
                for j, d in enumerate(dlist):
                    dg = diag_tile(d, sbldp)
                    nc.tensor.matmul(ps[d][:],
                                     hs[:, j * 128:(j + 1) * 128], dg[:],
                                     start=(total_d.get(d, 0) == 0),
                                     stop=True)
                    aggT = tailp.tile([128, 128], BF16, tag="aggT")
                    nc.scalar.activation(
                        aggT[:], ps[d][:],
                        mybir.ActivationFunctionType.Copy)
                    ps_o = psop.tile([128, 128], F32, tag="pso")
                    nc.tensor.matmul(ps_o[:], aggT[:], w2_s[:],
                                     start=True, stop=False)
                    if d in accT:
                        nc.tensor.matmul(ps_o[:], accT[d][:], w2_s[:],
                                         start=False, stop=False)
                    nc.tensor.matmul(ps_o[:], onesb_s[:], b2_s[:],
                                     start=False, stop=True)
                    nc.scalar.activation(ob2[:, j * 128:(j + 1) * 128],
                                         ps_o[:],
                                         mybir.ActivationFunctionType.Copy)
                lo = dlist[0] * 128
                hi = (dlist[-1] + 1) * 128
                nc.sync.dma_start(out_slab[:, lo:hi], ob2[:, :nsd * 128])
            wait_ctx.__exit__(None, None, None)

    nc.compile()
    return nc


def _pack_gix(gixf):
    """[C, NT*128] int16 -> [C, 128, NT*8] wrapped/replicated index layout."""
    C, n = gixf.shape
    NT = n // 128
    g = gixf.reshape(C, NT * 8, 16).transpose(0, 2, 1)
    return np.ascontiguousarray(np.tile(g, (1, 8, 1)))


def _pack_w(wf, dtype=np.float32):
    """[C, NT*128] -> [C, 128, NT]."""
    C, n = wf.shape
    NT = n // 128
    return np.ascontiguousarray(
        wf.reshape(C, NT, 128).transpose(0, 2, 1).astype(dtype))


def _cell_tables(sel, core, dtile, key_extra, n_extra, pad_idx, colv, nrmv):
    """Build packed per-core tables for edges selected by `sel`, grouped by
    cells = (key_extra, dtile) laid out in (sup, key_extra, dtile) order.

    Returns (t_counts [n_extra][TPC], gixf, colwf, nrmwf) where the flat
    arrays are [N_CORES, NT*128]."""
    core = core[sel]
    dtile = dtile[sel]
    ke = key_extra[sel]
    pad_idx = pad_idx[sel]
    colv = colv[sel]
    nrmv = nrmv[sel]

    sup = dtile // SUPER
    key = ((core * NSUP + sup) * n_extra + ke) * TPC + dtile
    order = np.argsort(key, kind="stable")
    kcd = (core * n_extra + ke) * TPC + dtile
    counts = np.bincount(kcd, minlength=N_CORES * n_extra * TPC)
    counts = counts.reshape(N_CORES, n_extra, TPC)
    t_counts = -(-counts.max(axis=0) // 128)       # [n_extra, TPC]

    slot_base = np.zeros((n_extra, TPC), np.int64)
    acc = 0
    for s in range(NSUP):
        for c in range(n_extra):
            for d in range(s * SUPER, min((s + 1) * SUPER, TPC)):
                slot_base[c, d] = acc
                acc += int(t_counts[c, d])
    NT = int(acc)

    key_s = key[order]
    group_start = np.concatenate(
        [[0], np.cumsum(np.bincount(key_s, minlength=key.max() + 1))[:-1]])
    rank = np.arange(len(key_s)) - group_start[key_s]

    gixf = np.zeros((N_CORES, NT * 128), np.int16)
    colwf = np.full((N_CORES, NT * 128), -1.0, np.float32)
    nrmwf = np.zeros((N_CORES, NT * 128), np.float32)
    pos = slot_base[ke[order], dtile[order]] * 128 + rank
    cidx = core[order]
    gixf[cidx, pos] = pad_idx[order].astype(np.int16)
    colwf[cidx, pos] = colv[order]
    nrmwf[cidx, pos] = nrmv[order]
    return t_counts, gixf, colwf, nrmwf


def _remote_tables(sel, core, dtile, prow, colv, nrmv, wkey=None, nwin=1,
                   pair=True):
    """Gather tables for one aggregation pass. With pair=True, `prow` is
    the row index within a packed fp8 table; the gather index is the PAIR
    prow>>1, parity prow&1 selects the 128B half of the 256B elem. With
    pair=False (layer 1), prow is gathered directly (single bf16 rows).
    Cells are (window, supertile); within a cell edges sort by (dtile[,
    parity]); tile-slot boundaries are uniform across cores (max-count
    padding), so the matmul slice list per tile is the cross-core union
    (pad slots have colw=-1, nrm=0 -> all-zero one-hot rows)."""
    core = core[sel]
    dtile = dtile[sel]
    if pair:
        pi = prow[sel] >> 1
        pr = prow[sel] & 1
    else:
        pi = prow[sel]
        pr = np.zeros(len(pi), np.int64)
    wk = (np.zeros(len(pi), np.int64) if wkey is None else wkey[sel])
    cv = colv[sel]
    nv = nrmv[sel]

    sup = dtile // SUPER
    key = (((core * NSUP + sup) * nwin + wk) * TPC + dtile) * 2 + pr
    order = np.argsort(key, kind="stable")
    cs = (core * NSUP + sup) * nwin + wk
    counts = np.bincount(cs, minlength=N_CORES * NSUP * nwin)
    counts = counts.reshape(N_CORES, NSUP * nwin)
    tcnt = (-(-counts.max(axis=0) // 128)).astype(np.int64)  # [NSUP*nwin]
    slot_base = np.concatenate([[0], np.cumsum(tcnt)[:-1]]).astype(np.int64)
    NT = int(tcnt.sum())

    cs_s = cs[order]
    grp = np.concatenate(
        [[0],
         np.cumsum(np.bincount(cs_s, minlength=N_CORES * NSUP * nwin))[:-1]])
    rank = np.arange(len(cs_s)) - grp[cs_s]
    tslot = slot_base[sup[order] * nwin + wk[order]] + rank // 128
    pos = tslot * 128 + rank % 128

    du = dtile[order]
    pu = pr[order]
    cu = core[order]
    d0 = np.full(NT, 1 << 30, np.int64)
    np.minimum.at(d0, tslot, du)
    doff = du - d0[tslot]
    assert len(doff) == 0 or doff.max() <= 3
    present = np.zeros(NT * 8, bool)
    present[(tslot * 4 + doff) * 2 + pu] = True
    stride = 2 if pair else 1
    colw2 = cv[order] + 128 * (stride * doff + pu)

    gixf = np.zeros((N_CORES, NT * 128), np.int16)
    colwf = np.full((N_CORES, NT * 128), -1.0, np.float32)
    nrmwf = np.zeros((N_CORES, NT * 128), np.float32)
    gixf[cu, pos] = pi[order].astype(np.int16)
    colwf[cu, pos] = colw2
    nrmwf[cu, pos] = nv[order]
    meta = tuple(
        (int(d0[t]) if d0[t] < (1 << 30) else 0,
         tuple((jj, p) for jj in range(4) for p in range(2)
               if present[(t * 4 + jj) * 2 + p]))
        for t in range(NT))
    seg = tuple(tuple(int(v) for v in tcnt[s * nwin:(s + 1) * nwin])
                for s in range(NSUP)) if nwin > 1 \
        else tuple(int(v) for v in tcnt)
    return seg, meta, gixf, colwf, nrmwf


def _preprocess(x, edge_index, edge_weight):
    """Host-side graph preprocessing -> uniform structure + per-core inputs."""
    row = np.asarray(edge_index[0], dtype=np.int64)
    col = np.asarray(edge_index[1], dtype=np.int64)
    ew = np.asarray(edge_weight, dtype=np.float32)
    n_nodes = N_NODES

    deg = np.bincount(col, weights=ew.astype(np.float64), minlength=n_nodes)
    deg = (deg + 1.0).astype(np.float32)
    dinv = (1.0 / np.sqrt(deg)).astype(np.float32)
    nrm = (dinv[row] * ew * dinv[col]).astype(np.float32)
    selfw_n = (dinv * dinv).astype(np.float32)

    core = col // NPC
    dtile = (col - core * NPC) >> 7
    colv = ((col - core * NPC) & 127).astype(np.float32)
    src_core = row // NPC
    pad_row = (src_core * PAD_NPC + (row - src_core * NPC)).astype(np.int64)
    chunk = np.searchsorted(np.asarray(CHUNK_LO), pad_row, side="right") - 1
    chunk_rel = pad_row - np.asarray(CHUNK_LO)[chunk]

    # layer 1: all edges, cells = (chunk window, supertile), single bf16 rows
    all_sel = np.ones(len(row), bool)
    seg1, meta1, gix1, colw1, nrmw1 = _remote_tables(
        all_sel, core, dtile, chunk_rel, colv, nrm,
        wkey=chunk, nwin=NCHUNK, pair=False)

    # layer 2: local edges (src on same core) gather the own fp8 slab;
    # remote edges split by source half: half A = rows [0, SPLIT_R) of each
    # slab (packed fp8 table from the first AllGather), half B = the rest.
    loc = src_core == core
    loc_idx = row - src_core * NPC           # local row within own slab
    rel = row - src_core * NPC               # local row within source slab
    in_a = rel < SPLIT_R
    rowA = src_core * SPLIT_R + rel          # row within fp8 table A
    rowB = src_core * (PAD_NPC - SPLIT_R) + (rel - SPLIT_R)
    segL, metaL, gixL, colwL, nrmwL = _remote_tables(
        loc, core, dtile, loc_idx, colv, nrm)
    segA, metaA, gixRA, colwRA, nrmwRA = _remote_tables(
        (~loc) & in_a, core, dtile, rowA, colv, nrm)
    segB, metaB, gixRB, colwRB, nrmwRB = _remote_tables(
        (~loc) & ~in_a, core, dtile, rowB, colv, nrm)

    selfw_pad = np.zeros(N_CORES * PAD_NPC, np.float32)
    idx_all = np.arange(n_nodes)
    c_all = idx_all // NPC
    selfw_pad[c_all * PAD_NPC + (idx_all - c_all * NPC)] = selfw_n
    selfw = np.ascontiguousarray(
        selfw_pad.reshape(N_CORES, TPC, 128).transpose(0, 2, 1))

    x = np.asarray(x, dtype=np.float32)
    x_tab = np.zeros((N_PAD, D), NPBF)
    x_tab.reshape(N_CORES, PAD_NPC, D)[:, :NPC, :] = \
        x.reshape(N_CORES, NPC, D).astype(NPBF)

    struct_key = (seg1, meta1, segL, metaL, segA, metaA, segB, metaB)
    tabs = dict(
        gix1=_pack_gix(gix1), colw1=_pack_w(colw1), nrmw1=_pack_w(nrmw1),
        gixRA=_pack_gix(gixRA), colwRA=_pack_w(colwRA),
        nrmwRA=_pack_w(nrmwRA),
        gixRB=_pack_gix(gixRB), colwRB=_pack_w(colwRB),
        nrmwRB=_pack_w(nrmwRB),
        gixL=_pack_gix(gixL), colwL=_pack_w(colwL), nrmwL=_pack_w(nrmwL),
        selfw=selfw, x_tab=x_tab)
    return struct_key, tabs


_NC_CACHE: dict = {}


def kernel(x, edge_index, edge_weight, W1, b1, W2, b2):
    struct_key, tabs = _preprocess(x, edge_index, edge_weight)

    if struct_key not in _NC_CACHE:
        seg1, meta1, segL, metaL, segA, metaA, segB, metaB = struct_key
        _NC_CACHE[struct_key] = _build_nc(
            [list(r) for r in seg1], list(meta1), list(segL), list(metaL),
            list(segA), list(metaA), list(segB), list(metaB))
    nc = _NC_CACHE[struct_key]

    w1_np = np.ascontiguousarray(np.asarray(W1, dtype=np.float32).astype(NPBF))
    w2_np = np.ascontiguousarray(np.asarray(W2, dtype=np.float32).astype(NPBF))
    b1_np = np.asarray(b1, dtype=np.float32).reshape(1, D).astype(NPBF)
    b2_np = np.asarray(b2, dtype=np.float32).reshape(1, D).astype(NPBF)
    iota = np.tile(np.arange(128), (128, 1)).astype(NPBF)
    iota_w = np.tile(np.arange(WMAX), (128, 1)).astype(np.float16)
    pidx = np.arange(128, dtype=np.float32).reshape(128, 1)
    ones = np.ones((1, D), np.float32)
    onesb = np.ones((1, D), NPBF)

    # partition-major self-row table: x_selT[p, d*128+f] = slab[d*128+p, f]
    x_selTs = np.ascontiguousarray(
        tabs["x_tab"].reshape(N_CORES, TPC, 128, D)
        .transpose(0, 2, 1, 3).reshape(N_CORES, 128, TPC * D))

    in_maps = []
    for c in range(N_CORES):
        m = {
            "x_selT": x_selTs[c],
            "x_tab": tabs["x_tab"],
            "selfw": tabs["selfw"][c], "pidx": pidx,
            "w1": w1_np, "b1": b1_np, "w2": w2_np, "b2": b2_np,
            "iota": iota, "iota_w": iota_w, "ones": ones, "onesb": onesb,
        }
        for k in ("gix1", "colw1", "nrmw1", "gixRA", "colwRA", "nrmwRA",
                  "gixRB", "colwRB", "nrmwRB", "gixL", "colwL", "nrmwL"):
            m[k] = tabs[k][c]
        in_maps.append(m)

    res = run_bass_kernel_spmd(nc, in_maps, core_ids=list(range(N_CORES)))
    # out_slab is partition-major bf16 [128, TPC*128]: row d*128+p is at
    # [p, d*128:...]; unpack per core and drop the 44 pad rows.
    out = np.concatenate(
        [np.asarray(res.results[c]["out_slab"])
         .reshape(128, TPC, D).transpose(1, 0, 2)
         .reshape(PAD_NPC, D)[:NPC].astype(np.float32)
         for c in range(N_CORES)], axis=0)
    return out
